# revision 19
# baseline (speedup 1.0000x reference)
"""ColAttention TRN2 kernel v3: out = gamma * colattn(x) + x.

Sharding: width. Core k gets x[:, :, :, 16k:16(k+1)]. Per core: 8 batches x 16
width columns = 128 independent attention problems over h=128.

Key design points vs baseline:
  - All I/O in fp16: x slab, a host-transposed x_t slab (B,WT,H,C), and the
    output (B,WT,H,C) -> halves HBM traffic; host up/down-casts.
  - Transposed scores: S^T(j,i) = matmul(lhsT=k_w, rhs=q_w)  (operand swap,
    fp16 n=128 full rate; baseline's f32r n=128 had a 4x PE penalty).
  - exp in bf16 (range-safe without max subtraction), no accumulator read.
  - Row sums via a 1-column ones-matmul sharing the exp^T weights; AV done as
    ONE n=512 matmul out[i, c] = ex_T.T @ V^T. The softmax normalizer 1/sums
    is per-PARTITION in this orientation, so it fuses into the final
    scalar_tensor_tensor: fin = (av * rr) + x_t  -> no PE transpose, no
    attn copy, no tensor_scalar normalize.
  - Final STT runs on the (otherwise idle) GpSimd/Pool engine; V^T PSUM->SBUF
    copies alternate ACT/DVE.
  - gamma folded into Wv on host; gamma*bv pre-added to x_t on host.
"""

import numpy as np
import ml_dtypes

import concourse.bass as bass
from concourse import bacc, mybir
from concourse.tile import TileContext
from concourse.bass_utils import run_bass_kernel_spmd

f32 = mybir.dt.float32
f16 = mybir.dt.float16
bf16 = mybir.dt.bfloat16
f8 = mybir.dt.float8e4
AF = mybir.ActivationFunctionType
ALU = mybir.AluOpType
DR = mybir.MatmulPerfMode.DoubleRow

FP8_SCALE = 256.0          # wv8 = e4m3(FP8_SCALE * gamma * Wv^T), hi+lo split

N_CORES = 8
B, C, H, W = 8, 512, 128, 128
WT = W // N_CORES          # 16 w-columns per core
DQ = 64
NCH = C // 128             # 4 c-chunks

TRACE = False              # set True from test.py for profiling
LAST_RESULTS = None


def _build():
    nc = bacc.Bacc("TRN2", num_devices=N_CORES, debug=False)

    x_d = nc.dram_tensor("x", (B, C, H, WT), f16, kind="ExternalInput")
    x8_d = nc.dram_tensor("x8", (B, C, H, WT), f8, kind="ExternalInput")
    xt_d = nc.dram_tensor("xt", (B, WT, H, C), f16, kind="ExternalInput")
    wqk_d = nc.dram_tensor("wqkT", (C, 128), f16, kind="ExternalInput")
    bqk_d = nc.dram_tensor("bqk", (64, 2), f32, kind="ExternalInput")
    # wv8: SBUF-layout (128, 4096): [g(2) | hi/lo(2) | ci-in-pair(2) | 512]
    wv8_d = nc.dram_tensor("wv8", (128, 4096), f8, kind="ExternalInput")
    out_d = nc.dram_tensor("out", (B, WT, H, C), f16, kind="ExternalOutput")
    ones_d = nc.inline_tensor(np.ones((128, 1), dtype=ml_dtypes.bfloat16),
                              name="ones128")

    xa = x_d.ap()
    x8a = x8_d.ap()
    xta = xt_d.ap()
    oa = out_d.ap()

    with TileContext(nc) as tc:
        with (
            tc.tile_pool(name="const", bufs=1) as cpool,
            tc.tile_pool(name="xs", bufs=2) as xspool,
            tc.tile_pool(name="x8", bufs=2) as x8pool,
            tc.tile_pool(name="xt", bufs=2) as xtpool,
            tc.tile_pool(name="qk", bufs=2) as qkpool,
            tc.tile_pool(name="small", bufs=3) as spool,
            tc.tile_pool(name="fin", bufs=2) as fpool,
            tc.tile_pool(name="pqk", bufs=1, space="PSUM") as pqk,
            tc.tile_pool(name="pvt", bufs=2, space="PSUM") as pvt,
            tc.tile_pool(name="psc", bufs=2, space="PSUM") as psc,
            tc.tile_pool(name="pav", bufs=2, space="PSUM") as pav,
        ):
            # ---- constants ----
            wqk_sb = cpool.tile([128, 128 * NCH], f16, name="wqk_sb")
            for ci in range(NCH):
                nc.sync.dma_start(wqk_sb[:, ci * 128:(ci + 1) * 128],
                                  wqk_d.ap()[ci * 128:(ci + 1) * 128, :])
            wv_sb = cpool.tile([128, 4096], f8, name="wv_sb")
            nc.sync.dma_start(wv_sb[:], wv8_d.ap())
            bqk_sb = cpool.tile([64, 2], f32, name="bqk_sb")
            nc.sync.dma_start(bqk_sb[:], bqk_d.ap())
            ones_sb = cpool.tile([128, 1], bf16, name="ones_sb")
            nc.sync.dma_start(ones_sb[:], ones_d.ap())

            # one-deep software pipeline state: (b, w, ex, v_sb, xt3, fin4)
            pend = None

            def stage2(pb, w, ex, v_sb, sc, xt3, fin4):
                # row sums via ones-matmul (same weights as AV); result lands
                # in the spare column of the (still-live) score tile's bank
                sm = sc[:, 128:129]
                nc.tensor.matmul(sm, ex[:], ones_sb[:], start=True, stop=True)
                # AV: out[i, c] = sum_j ex_T(j,i) * V^T(j,c)   (one n=512 mm)
                av = pav.tile([128, 512], f32, tag="av")
                nc.tensor.matmul(av[:], ex[:], v_sb[:], start=True, stop=True)
                rr = spool.tile([128, 1], f32, tag="rr")
                nc.vector.reciprocal(rr[:], sm)
                # normalize: av_s = av * rr (PSUM drain, DVE/ACT alternate),
                # then residual add on GpSimd (SBUF-only engine)
                av_s = spool.tile([128, 512], f16, tag="avs")
                if w % 2 == 0:
                    nc.scalar.activation(av_s[:], av[:], AF.Identity, scale=rr[:])
                else:
                    nc.vector.tensor_scalar_mul(av_s[:], av[:], rr[:])
                nc.gpsimd.tensor_add(
                    fin4[:, (w % 4) * 512:(w % 4 + 1) * 512],
                    av_s[:], xt3[:, w])
                if w % 4 == 3:
                    nc.sync.dma_start(
                        oa[pb, w - 3:w + 1].rearrange("w h c -> h w c"),
                        fin4[:].rearrange("p (w c) -> p w c", c=512))

            for b in range(B):
                # ---- batch prologue: hoisted into previous batch's w-loop ----
                with tc.high_priority(offset=0 if b == 0 else 200):
                    xs = xspool.tile([128, NCH * H * WT], f16, tag="xs",
                                     name=f"xs{b}")
                    xs4 = xs[:].rearrange("p (c h w) -> p c h w", c=NCH, w=WT)
                    for ci in range(NCH):
                        nc.sync.dma_start(xs4[:, ci], xa[b, ci * 128:(ci + 1) * 128])
                    xs8 = x8pool.tile([128, NCH * H * WT], f8, tag="xs8",
                                      name=f"xs8{b}")
                    xs84 = xs8[:].rearrange("p (c h w) -> p c h w", c=NCH, w=WT)
                    for ci in range(NCH):
                        nc.sync.dma_start(xs84[:, ci],
                                          x8a[b, ci * 128:(ci + 1) * 128])

                    xt_sb = xtpool.tile([128, WT * 512], f16, tag="xt",
                                        name=f"xt{b}")
                    for ld in range(4):
                        nc.sync.dma_start(
                            xt_sb[:, ld * 2048:(ld + 1) * 2048].rearrange(
                                "p (w c) -> p w c", c=512),
                            xta[b, ld * 4:(ld + 1) * 4].rearrange("w h c -> h w c"))

                    # QK projection -> q (64 parts) and k (64 parts, shifted)
                    q_sb = qkpool.tile([64, H * WT], f16, tag="q", name=f"q{b}")
                    k_sb = qkpool.tile([64, H * WT], f16, tag="k", name=f"k{b}")
                    for nt in range(H * WT // 512):
                        qkp = pqk.tile([128, 512], f32, tag="qkp")
                        for ci in range(NCH):
                            nc.tensor.matmul(
                                qkp[:],
                                wqk_sb[:, ci * 128:(ci + 1) * 128],
                                xs[:, ci * 2048 + nt * 512: ci * 2048 + (nt + 1) * 512],
                                start=(ci == 0), stop=(ci == NCH - 1))
                        nc.scalar.activation(q_sb[:, nt * 512:(nt + 1) * 512],
                                             qkp[0:64, :], AF.Identity,
                                             bias=bqk_sb[:, 0:1])
                        nc.scalar.activation(k_sb[:, nt * 512:(nt + 1) * 512],
                                             qkp[64:128, :], AF.Identity,
                                             bias=bqk_sb[:, 1:2])
                q3 = q_sb[:].rearrange("p (h w) -> p h w", w=WT)
                k3 = k_sb[:].rearrange("p (h w) -> p h w", w=WT)
                xt3 = xt_sb[:].rearrange("p (w c) -> p w c", c=512)

                fin4 = None
                for w in range(WT):
                    # ---- stage 1: V^T (fp8 DoubleRow), scores^T, exp ----
                    # vt = sum_g sum_{hi,lo} x8[2g:2g+2].T @ wv8[g, hl]
                    # (k-chunk pair packed in the DoubleRow t-slot)
                    vt = pvt.tile([128, 512], f32, tag="vt")
                    for gi in range(2):
                        lhs = xs84[:, 2 * gi:2 * gi + 2, :, w]
                        for hl in range(2):
                            nc.tensor.matmul(
                                vt[:], lhs,
                                wv_sb[:, gi * 2048 + hl * 1024:
                                      gi * 2048 + (hl + 1) * 1024].rearrange(
                                          "p (t n) -> p t n", t=2),
                                start=(gi == 0 and hl == 0),
                                stop=(gi == 1 and hl == 1),
                                perf_mode=DR)
                    v_sb = spool.tile([128, 512], bf16, tag="v_sb")
                    if w % 2 == 0:
                        nc.vector.tensor_scalar_mul(v_sb[:], vt[:],
                                                    1.0 / FP8_SCALE)
                    else:
                        nc.scalar.activation(v_sb[:], vt[:], AF.Identity,
                                             scale=1.0 / FP8_SCALE)

                    # S^T(j, i): lhsT = k_w, rhs = q_w  (fp16, n=128)
                    sc = psc.tile([128, 132], f32, tag="sc")
                    nc.tensor.matmul(sc[:, 0:128], k3[:, :, w], q3[:, :, w],
                                     start=True, stop=True)
                    ex = spool.tile([128, 128], bf16, tag="ex")
                    nc.scalar.activation(ex[:], sc[:, 0:128], AF.Exp)

                    if w % 4 == 0:
                        fin4 = fpool.tile([128, 2048], f16, tag="fin",
                                          name=f"fin{b}_{w}")
                    if pend is not None:
                        stage2(*pend)
                    pend = (b, w, ex, v_sb, sc, xt3, fin4)

            stage2(*pend)

    nc.compile()
    return nc


def kernel(x, Wq, bq, Wk, bk, Wv, bv, gamma):
    global LAST_RESULTS
    x = np.asarray(x, dtype=np.float32)
    Wq = np.asarray(Wq, dtype=np.float32)
    bq = np.asarray(bq, dtype=np.float32)
    Wk = np.asarray(Wk, dtype=np.float32)
    bk = np.asarray(bk, dtype=np.float32)
    Wv = np.asarray(Wv, dtype=np.float32)
    bv = np.asarray(bv, dtype=np.float32)
    g = float(np.asarray(gamma, dtype=np.float32).reshape(-1)[0])

    nc = _build()

    wqkT = np.ascontiguousarray(
        np.concatenate([Wq, Wk], axis=0).T).astype(np.float16)        # (C, 128)
    bqk = np.ascontiguousarray(
        np.stack([bq, bk], axis=1)).astype(np.float32)                # (64, 2)
    gbv = (g * bv).astype(np.float32)                                 # (C,)

    # fp8 V weights: hi+lo split of FP8_SCALE * (g*Wv)^T, packed in the
    # SBUF layout [g(2) | hi/lo(2) | ci-in-pair(2) | 512] on 128 partitions
    wvT = np.ascontiguousarray((g * Wv).T).astype(np.float32)         # (C, C)
    whi = (FP8_SCALE * wvT).astype(ml_dtypes.float8_e4m3fn)
    wlo = (FP8_SCALE * wvT - whi.astype(np.float32)).astype(
        ml_dtypes.float8_e4m3fn)
    wv8 = np.empty((128, 4096), dtype=ml_dtypes.float8_e4m3fn)
    for gi in range(2):
        for hl, wsrc in enumerate((whi, wlo)):
            for cj in range(2):
                ci = 2 * gi + cj
                wv8[:, gi * 2048 + hl * 1024 + cj * 512:
                    gi * 2048 + hl * 1024 + (cj + 1) * 512] = \
                    wsrc[ci * 128:(ci + 1) * 128, :]

    x16 = x.astype(np.float16)
    x8 = x16.astype(ml_dtypes.float8_e4m3fn)
    in_maps = []
    for k in range(N_CORES):
        sl = x[:, :, :, k * WT:(k + 1) * WT]
        xt = sl.transpose(0, 3, 2, 1) + gbv[None, None, None, :]      # (B,WT,H,C)
        in_maps.append({
            "x": np.ascontiguousarray(x16[:, :, :, k * WT:(k + 1) * WT]),
            "x8": np.ascontiguousarray(x8[:, :, :, k * WT:(k + 1) * WT]),
            "xt": np.ascontiguousarray(xt.astype(np.float16)),
            "wqkT": wqkT,
            "bqk": bqk,
            "wv8": wv8,
        })

    res = run_bass_kernel_spmd(nc, in_maps, core_ids=list(range(N_CORES)),
                               trace=TRACE)
    LAST_RESULTS = res

    out = np.empty((B, C, H, W), dtype=np.float32)
    for k in range(N_CORES):
        out[:, :, :, k * WT:(k + 1) * WT] = \
            res.results[k]["out"].transpose(0, 3, 2, 1).astype(np.float32)
    return out
